# revision 1
# baseline (speedup 1.0000x reference)
"""AttentionHeadCheb distributed Trainium2 kernel (8 NeuronCores).

Destination-node sharding; 2 source-chunk phases; per (row,chunk) runs
padded to x4; packed into 16 segs of 4096 edges per phase (4 reserved pad
edges per seg; groups never straddle segs). Blocks = 2 segs (8192 edges,
one ap_gather group each). Tables (f32) time-share one SBUF slot:
al -> ar -> wx per phase.
"""

import numpy as np
import ml_dtypes

BF16 = ml_dtypes.bfloat16

N_NODES = 50000
IN_DIM = 128
OUT_DIM = 64
NC = 8
NLOC = N_NODES // NC          # 6250
CHUNK = N_NODES // 2          # 25000
W4 = 4
SEG = 4096                    # packing unit (reserved 4 pads at start)
BLK = 8192                    # edges per block = ap_gather group
NBLK = 8
TPH = BLK * NBLK              # 65536
PBLK = BLK // W4              # 2048 partials per block
PQTR = 4096                   # partials per quarter (2 blocks)
NLE = 6256                    # NLOC padded to x16
BIAS_PAD = -60.0
SUBA = 512                    # alar sub-tile
SUBM = 1024                   # main sub-tile (= 256 partials)


def _pack_weights(W_transform, w_left, w_right, W_residual):
    W01 = np.concatenate([W_transform[0], W_transform[1]], axis=1)
    LAL = np.zeros((128, 128), np.float32)
    LAR = np.zeros((128, 16), np.float32)
    for i in range(3):
        LAL[0:64, i::16] = w_left[0][i][:, None]
        LAL[64:128, (4 + i)::16] = w_left[1][i][:, None]
        LAR[0:64, i] = w_right[0][i]
        LAR[64:128, 4 + i] = w_right[1][i]
    WRT = W_residual[0:IN_DIM]
    WRB = np.concatenate([W_residual[IN_DIM:], W_residual[IN_DIM:]], axis=0)
    DSEL = np.zeros((128, 2), np.float32)
    DSEL[0::16, 0] = 1.0
    DSEL[4::16, 1] = 1.0
    # CSEL[K, p] = 1 iff out-row p (= 16g+4k) sums partitions 16g+4k..+3
    CSEL = np.zeros((128, 128), np.float32)
    for g in range(8):
        for k in range(2):
            CSEL[16 * g + 4 * k:16 * g + 4 * k + 4, 16 * g + 4 * k] = 1.0
    return (W01.astype(BF16), LAL.astype(BF16), LAR.astype(BF16),
            WRT.astype(BF16), WRB.astype(BF16), DSEL.astype(np.float32),
            CSEL.astype(BF16))


def _wrap16_rep(vals, nidx):
    v = vals.reshape(nidx // 16, 16).T
    return np.tile(v, (8, 1)).astype(np.int16)


def _wrap16_grouped(vals):
    g, eb = vals.shape
    out = np.empty((16 * g, eb // 16), np.int16)
    for gg in range(g):
        out[16 * gg:16 * gg + 16] = vals[gg].reshape(eb // 16, 16).T
    return out


def _prep_core(m, r, c, atten_vals, support_vals):
    sel = np.where((r >= m * NLOC) & (r < (m + 1) * NLOC))[0]
    rl = (r[sel] - m * NLOC).astype(np.int64)
    cg = c[sel].astype(np.int64)
    ch = (cg // CHUNK).astype(np.int64)
    order = np.lexsort((ch, rl))
    sel, rl, cg, ch = sel[order], rl[order], cg[order], ch[order]
    cl = (cg % CHUNK).astype(np.int64)

    phases = []
    for pc in (0, 1):
        pm = ch == pc
        prl, pcl, psel = rl[pm], cl[pm], sel[pm]
        ne = prl.size
        gstart = np.flatnonzero(np.r_[True, prl[1:] != prl[:-1]]) if ne else \
            np.zeros(0, np.int64)
        gcnt = np.diff(np.r_[gstart, ne]) if ne else np.zeros(0, np.int64)
        grow = prl[gstart] if ne else np.zeros(0, np.int64)
        gpad = ((gcnt + 3) // 4) * 4
        ng = grow.size
        gpos = np.empty(ng, np.int64)       # global slot of group start
        seg_i, off = 0, 4
        NSEG = TPH // SEG
        for i in range(ng):
            if off + gpad[i] > SEG:
                seg_i += 1
                off = 4
            assert seg_i < NSEG, f"core {m} phase {pc}: seg overflow"
            gpos[i] = seg_i * SEG + off
            off += gpad[i]
        within = np.arange(ne) - np.repeat(gstart, gcnt)
        slot = np.repeat(gpos, gcnt) + within
        cols = np.zeros(TPH, np.int64)
        rows = np.zeros(TPH, np.int64)
        vrow = np.zeros((8, TPH), np.float32)
        vrow[3] = BIAS_PAD
        vrow[7] = BIAS_PAD
        cols[slot] = pcl
        rows[slot] = prl
        e0 = psel
        vrow[0][slot] = atten_vals[0][e0]
        vrow[1][slot] = atten_vals[1][e0]
        vrow[2][slot] = support_vals[0][e0]
        vrow[3][slot] = 0.0
        vrow[4][slot] = atten_vals[0][e0]
        vrow[5][slot] = atten_vals[1][e0]
        vrow[6][slot] = support_vals[1][e0]
        vrow[7][slot] = 0.0
        # edge segment ids (pads negative per seg)
        esid = np.zeros(TPH, np.int64)
        for si in range(NSEG):
            esid[si * SEG:(si + 1) * SEG] = -(si + 1)
        gp_hi = gpos + gpad
        for i in range(ng):
            esid[gpos[i]:gp_hi[i]] = i
        emask = np.ones(TPH, np.float32)
        emask[0] = 0.0
        emask[1:][esid[1:] != esid[:-1]] = 0.0
        emask[0::SEG] = 0.0
        psid = esid[0::W4]
        pmask = np.ones(TPH // W4, np.float32)
        pmask[0] = 0.0
        pmask[1:][psid[1:] != psid[:-1]] = 0.0
        pmask[0::SEG // W4] = 0.0
        # msgs ends per quarter (quarter = 4096 partials = 16384 edges)
        pend = gp_hi // W4 - 1
        endq = np.zeros((4, NLE), np.int64)
        gq = pend // PQTR
        for i in range(ng):
            endq[gq[i], grow[i]] = pend[i] - gq[i] * PQTR
        # denom ends per col-half of each block (exs table [*, 4096])
        dend = np.zeros((2, 8, NLE), np.int64)   # [half, group, row]
        gblk = gpos // BLK
        eloc = (gp_hi - 1) % BLK                 # block-local end edge
        for i in range(ng):
            h = eloc[i] // 4096
            dend[h, gblk[i], grow[i]] = eloc[i] - h * 4096
        colw = np.empty((128, TPH // 16), np.int16)
        for t in range(NBLK):
            colw[:, t * (BLK // 16):(t + 1) * (BLK // 16)] = _wrap16_rep(
                cols[t * BLK:(t + 1) * BLK], BLK)
        rloc = _wrap16_grouped(rows.reshape(8, BLK))
        cloc = _wrap16_grouped(cols.reshape(8, BLK))
        endqw = [_wrap16_rep(endq[q], NLE) for q in range(4)]
        dendw = [_wrap16_grouped(dend[h]) for h in range(2)]
        vst = np.zeros((128, BLK), BF16)
        for g in range(8):
            for i in range(8):
                vst[16 * g + i] = vrow[i][g * BLK:(g + 1) * BLK].astype(BF16)
        emask8 = emask.reshape(8, BLK).astype(BF16)
        pmask8 = pmask.reshape(8, PBLK).astype(BF16)
        pmrep = np.broadcast_to(pmask.astype(BF16)[None, :],
                                (128, TPH // W4)).copy()
        emrep = np.repeat(emask.reshape(8, BLK).astype(BF16), 16, axis=0)
        ph = dict(colw=colw, rloc=rloc, cloc=cloc, emask=emask8,
                  pmask=pmask8, pmrep=pmrep, emrep=emrep, vst=vst)
        for q in range(4):
            ph[f"endq{q}"] = endqw[q]
        for h in range(2):
            ph[f"dendw{h}"] = dendw[h]
        phases.append(ph)
    return phases


def host_prep(x, support_vals, atten_vals, W_transform, w_left, w_right,
              W_residual, edge_rows, edge_cols):
    W01, LAL, LAR, WRT, WRB, DSEL, CSEL = _pack_weights(
        W_transform, w_left, w_right, W_residual)
    ONESROW = np.ones((1, NLOC), np.float32)
    in_maps = []
    for m in range(NC):
        ph = _prep_core(m, edge_rows, edge_cols, atten_vals, support_vals)
        xT = np.ascontiguousarray(x[m * NLOC:(m + 1) * NLOC].T).astype(BF16)
        im = dict(xT=xT, W01=W01, LAL=LAL, LAR=LAR, WRT=WRT, WRB=WRB,
                  DSEL=DSEL, CSEL=CSEL, ONESROW=ONESROW)
        for pc in (0, 1):
            for k, v in ph[pc].items():
                im[f"{k}{pc}"] = np.ascontiguousarray(v)
        in_maps.append(im)
    return in_maps


# ======================================================================
# Numpy emulation
# ======================================================================

def emulate(in_maps, x, W_transform, w_left, w_right, W_residual):
    xb = x.astype(BF16).astype(np.float32)
    wx_all = np.concatenate(
        [xb @ W_transform[k].astype(BF16).astype(np.float32)
         for k in range(2)], axis=1)
    wxT = wx_all.T
    ar_all = np.zeros((16, N_NODES), np.float32)
    al_all = np.zeros((16, N_NODES), np.float32)
    for k in range(2):
        ar_all[4 * k:4 * k + 3] = (wx_all[:, 64 * k:64 * k + 64] @
                                   w_right[k].T).T
        al_all[4 * k:4 * k + 3] = (wx_all[:, 64 * k:64 * k + 64] @
                                   w_left[k].T).T
    al_all[3] = 1.0
    al_all[7] = 1.0

    def segscan(parts, mrow):
        cs = np.cumsum(parts, axis=-1)
        starts = np.flatnonzero(mrow == 0.0)
        seg = np.cumsum(mrow == 0.0) - 1
        offs = np.take(cs[..., starts] - parts[..., starts], seg, axis=-1)
        return cs - offs

    outs = []
    for m in range(NC):
        im = in_maps[m]
        al_loc = al_all[:, m * NLOC:(m + 1) * NLOC]
        msum = np.zeros((128, NLOC), np.float64)
        dsum = np.zeros((2, NLOC), np.float64)
        for pc in (0, 1):
            rloc = im[f"rloc{pc}"].astype(np.int64)
            cloc = im[f"cloc{pc}"].astype(np.int64)
            alo = np.zeros((128, BLK), np.float32)
            aro = np.zeros((128, BLK), np.float32)
            for g in range(8):
                idx = rloc[16 * g:16 * g + 16].T.reshape(-1)
                alo[16 * g:16 * g + 16] = al_loc[:, idx]
                idxc = cloc[16 * g:16 * g + 16].T.reshape(-1)
                aro[16 * g:16 * g + 16] = ar_all[:, pc * CHUNK + idxc]
            alo = alo.astype(BF16).astype(np.float32)   # alv compress
            p8 = (alo + aro) * im[f"vst{pc}"].astype(np.float32)
            s = p8[0::4] + p8[1::4] + p8[2::4] + p8[3::4]
            ex8 = np.exp(s)
            emask = im[f"emask{pc}"].astype(np.float32)
            exs = np.zeros((32, BLK), np.float32)
            for g in range(8):
                for k in (0, 1):
                    for h in (0, 1):
                        sl = slice(h * 4096, (h + 1) * 4096)
                        exs[4 * g + k, sl] = segscan(ex8[4 * g + k, sl],
                                                     emask[g, sl])
            for h in (0, 1):
                dendw = im[f"dendw{h}{pc}"].astype(np.int64)
                for g in range(8):
                    idx = dendw[16 * g:16 * g + 16].T.reshape(-1)
                    dsum[0] += exs[4 * g + 0, h * 4096 + idx][:NLOC]
                    dsum[1] += exs[4 * g + 1, h * 4096 + idx][:NLOC]
            colw = im[f"colw{pc}"].astype(np.int64)
            cols = np.empty(TPH, np.int64)
            for t in range(NBLK):
                blkw = colw[0:16, t * 512:(t + 1) * 512]
                cols[t * BLK:(t + 1) * BLK] = blkw.T.reshape(-1)
            pmask = im[f"pmask{pc}"].astype(np.float32)
            for q in range(4):
                scanq = np.zeros((128, PQTR), np.float32)
                for tt in range(2):
                    t = 2 * q + tt
                    idx = pc * CHUNK + cols[t * BLK:(t + 1) * BLK]
                    g = wxT[:, idx]
                    g = g * np.where((np.arange(128) < 64)[:, None],
                                     ex8[4 * t + 0], ex8[4 * t + 1])
                    part = g.reshape(128, PBLK, W4).sum(2)
                    # device scans per 512-partial sub chained within seg
                    # (= segscan with pmask, segs break at seg bounds)
                    scanq[:, tt * PBLK:(tt + 1) * PBLK] = segscan(
                        part, pmask[t])
                endw = im[f"endq{q}{pc}"].astype(np.int64)
                eidx = endw[0:16].T.reshape(-1)
                msum += scanq[:, eidx[:NLOC]]
        dsum += 1e-30
        out01 = msum.copy()
        out01[0:64] /= dsum[0]
        out01[64:128] /= dsum[1]
        xs = xb[m * NLOC:(m + 1) * NLOC]
        pre = (xs @ W_residual[:IN_DIM] +
               (out01[0:64] + out01[64:128]).T @ W_residual[IN_DIM:])
        out = np.where(pre > 0, pre, np.exp(np.minimum(pre, 0)) - 1)
        outs.append(out.astype(np.float32))
    return np.concatenate(outs, axis=0)


# ======================================================================
# Bass kernel builder
# ======================================================================

def build_bass():
    import sys
    if '/opt/trn_rl_repo' not in sys.path:
        sys.path.insert(0, '/opt/trn_rl_repo')
    from concourse import bass, bacc, tile, mybir

    dt = mybir.dt
    AL = mybir.AluOpType
    AF = mybir.ActivationFunctionType

    nc = bacc.Bacc(None, target_bir_lowering=False)

    def din(name, shape, d):
        return nc.dram_tensor(name, list(shape), d, kind="ExternalInput")

    xT_d = din("xT", (128, NLOC), dt.bfloat16)
    W01_d = din("W01", (128, 128), dt.bfloat16)
    LAL_d = din("LAL", (128, 128), dt.bfloat16)
    LAR_d = din("LAR", (128, 16), dt.bfloat16)
    WRT_d = din("WRT", (128, 64), dt.bfloat16)
    WRB_d = din("WRB", (128, 64), dt.bfloat16)
    DSEL_d = din("DSEL", (128, 2), dt.float32)
    CSEL_d = din("CSEL", (128, 128), dt.bfloat16)
    ONESROW_d = din("ONESROW", (1, NLOC), dt.float32)
    ph_d = []
    for pc in (0, 1):
        dd = dict(
            colw=din(f"colw{pc}", (128, TPH // 16), dt.int16),
            rloc=din(f"rloc{pc}", (128, BLK // 16), dt.int16),
            cloc=din(f"cloc{pc}", (128, BLK // 16), dt.int16),
            emask=din(f"emask{pc}", (8, BLK), dt.bfloat16),
            pmask=din(f"pmask{pc}", (8, PBLK), dt.bfloat16),
            vst=din(f"vst{pc}", (128, BLK), dt.bfloat16),
            pmrep=din(f"pmrep{pc}", (128, TPH // W4), dt.bfloat16),
            emrep=din(f"emrep{pc}", (128, BLK), dt.bfloat16),
        )
        for q in range(4):
            dd[f"endq{q}"] = din(f"endq{q}{pc}", (128, NLE // 16), dt.int16)
        for h in range(2):
            dd[f"dendw{h}"] = din(f"dendw{h}{pc}", (128, NLE // 16), dt.int16)
        ph_d.append(dd)
    out_d = nc.dram_tensor("out", [64, NLOC], dt.float32,
                           kind="ExternalOutput")
    agin = nc.dram_tensor("agin", [144, NLOC], dt.float32)
    agout = nc.dram_tensor("agout", [144 * NC, NLOC], dt.float32,
                           addr_space="Shared")
    al_dram = nc.dram_tensor("al_stash", [128, NLOC], dt.float32)

    NT512 = (NLOC + 511) // 512
    NJ = (NLE + 511) // 512

    with tile.TileContext(nc) as tc:
      with nc.allow_low_precision(reason="bf16 accums validated in emulation"):
        with (
            tc.tile_pool(name="big", bufs=1) as big,
            tc.tile_pool(name="res", bufs=1) as res,
            tc.tile_pool(name="mid", bufs=1) as mid,
            tc.tile_pool(name="work", bufs=2) as work,
            tc.tile_pool(name="work1", bufs=1) as work1,
            tc.tile_pool(name="psum", bufs=2, space="PSUM") as psum,
            tc.tile_pool(name="psum1", bufs=1, space="PSUM") as psum1,
        ):
            # ---------- stage 1 ----------
            xT = res.tile([128, NLOC], dt.bfloat16, tag="alv")
            nc.sync.dma_start(xT[:], xT_d[:])
            W01 = mid.tile([128, 128], dt.bfloat16, tag="w128")
            nc.sync.dma_start(W01[:], W01_d[:])
            stash = big.tile([128, CHUNK], dt.float32, tag="big")
            wx_own = stash[:, 0:NLOC]
            al8r = stash[:, NLOC:2 * NLOC]
            for j in range(NT512):
                a, b = j * 512, min(NLOC, (j + 1) * 512)
                pw = psum.tile([128, 512], dt.float32, tag="pw")
                nc.tensor.matmul(pw[:, :b - a], W01[:], xT[:, a:b],
                                 start=True, stop=True)
                nc.vector.tensor_copy(wx_own[:, a:b], pw[:, :b - a])
            LALt = mid.tile([128, 128], dt.bfloat16, tag="w128b")
            LARt = mid.tile([128, 16], dt.bfloat16, tag="w16")
            nc.sync.dma_start(LALt[:], LAL_d[:])
            nc.sync.dma_start(LARt[:], LAR_d[:])
            wxb = res.tile([128, NLOC], dt.bfloat16, tag="ex8")
            nc.vector.tensor_copy(wxb[:], wx_own[:])
            for j in range(NT512):
                a, b = j * 512, min(NLOC, (j + 1) * 512)
                pa = psum.tile([128, 512], dt.float32, tag="pw")
                nc.tensor.matmul(pa[:, :b - a], LALt[:], wxb[:, a:b],
                                 start=True, stop=True)
                nc.vector.tensor_copy(al8r[:, a:b], pa[:, :b - a])
                pr = psum.tile([16, 512], dt.float32, tag="pw")
                nc.tensor.matmul(pr[:, :b - a], LARt[:], wxb[:, a:b],
                                 start=True, stop=True)
                ar16s = work1.tile([16, 512], dt.float32, tag="alo")
                nc.scalar.activation(ar16s[:, :b - a], pr[:, :b - a], AF.Copy)
                nc.sync.dma_start(agin[128:144, a:b], ar16s[:, :b - a])
            for g8 in range(8):
                nc.sync.dma_start(al8r[16 * g8 + 3:16 * g8 + 4, :],
                                  ONESROW_d[:])
                nc.sync.dma_start(al8r[16 * g8 + 7:16 * g8 + 8, :],
                                  ONESROW_d[:])
            nc.sync.dma_start(al_dram[:], al8r[:])
            nc.sync.dma_start(agin[0:128, :], wx_own[:])
            nc.gpsimd.collective_compute(
                "AllGather", AL.bypass,
                replica_groups=[list(range(NC))],
                ins=[agin.ap().opt()],
                outs=[agout.ap().opt()],
            )

            msum = res.tile([128, NLOC], dt.bfloat16, tag="msum")
            dsum = mid.tile([2, NLE], dt.bfloat16, tag="dsum")
            nc.vector.memset(dsum[:], 0.0)
            nc.vector.memset(msum[:], 0.0)
            DSELt = mid.tile([128, 2], dt.float32, tag="dsel")
            nc.sync.dma_start(DSELt[:], DSEL_d[:])
            CSELt = mid.tile([128, 128], dt.bfloat16, tag="csel")
            nc.sync.dma_start(CSELt[:], CSEL_d[:])
            ones65 = mid.tile([65, 64], dt.bfloat16, tag="ones1")
            nc.vector.memset(ones65[0:1, :], 1.0)
            nc.vector.memset(ones65[64:65, :], 1.0)

            for pc in (0, 1):
                pd = ph_d[pc]
                rloc = mid.tile([128, BLK // 16], dt.int16, tag="rloc")
                cloc = mid.tile([128, BLK // 16], dt.int16, tag="cloc")
                nc.sync.dma_start(rloc[:], pd["rloc"][:])
                nc.sync.dma_start(cloc[:], pd["cloc"][:])
                # --- A: al gather ---
                altab = big.tile([128, CHUNK], dt.float32, tag="big")
                nc.sync.dma_start(altab[:, 0:NLOC], al_dram[:])
                alv = res.tile([128, BLK], dt.bfloat16, tag="alv")
                for s in range(BLK // SUBA):
                    sw = SUBA // 16
                    alo = work1.tile([128, SUBA], dt.float32, tag="alo")
                    nc.gpsimd.ap_gather(alo[:], altab[:, 0:NLOC],
                                        rloc[:, s * sw:(s + 1) * sw],
                                        channels=128, num_elems=NLOC, d=1,
                                        num_idxs=SUBA)
                    nc.vector.tensor_copy(alv[:, s * SUBA:(s + 1) * SUBA],
                                          alo[:])
                # --- B: ar gather + scores + denom ---
                artab = big.tile([128, CHUNK], dt.float32, tag="big")
                for q in range(4):
                    rk = 4 * pc + q
                    for g in range(8):
                        nc.sync.dma_start(
                            artab[16 * g:16 * g + 16,
                                  q * NLOC:(q + 1) * NLOC],
                            agout[rk * 144 + 128:rk * 144 + 144, :])
                ex8 = res.tile([128, BLK], dt.bfloat16, tag="ex8")
                for s in range(BLK // SUBA):
                    a, b = s * SUBA, (s + 1) * SUBA
                    sw = SUBA // 16
                    aro = work1.tile([128, SUBA], dt.float32, tag="alo")
                    nc.gpsimd.ap_gather(aro[:], artab[:],
                                        cloc[:, s * sw:(s + 1) * sw],
                                        channels=128, num_elems=CHUNK, d=1,
                                        num_idxs=SUBA)
                    vsts = work1.tile([128, SUBA], dt.bfloat16, tag="vsts")
                    nc.sync.dma_start(vsts[:], pd["vst"][:, a:b])
                    p8 = work1.tile([128, SUBA], dt.bfloat16, tag="p8")
                    nc.vector.tensor_tensor(p8[:], aro[:], alv[:, a:b],
                                            AL.add)
                    nc.vector.tensor_tensor(p8[:], p8[:], vsts[:],
                                            AL.mult)
                    sxp = psum.tile([128, SUBA], dt.float32, tag="pw")
                    nc.tensor.matmul(sxp[:], CSELt[:], p8[:],
                                     start=True, stop=True)
                    nc.scalar.activation(ex8[:, a:b], sxp[:], AF.Exp)
                emaskh = mid.tile([128, 4096], dt.bfloat16, tag="emaskh")
                for h in (0, 1):
                    nc.sync.dma_start(emaskh[:],
                                      pd["emrep"][:, h * 4096:(h + 1) * 4096])
                    exs = res.tile([128, 4096], dt.float32, tag="sh16")
                    hs = slice(h * 4096, (h + 1) * 4096)
                    nc.vector.tensor_tensor_scan(
                        exs[:, :], emaskh[:], ex8[:, hs], 0.0,
                        op0=AL.mult, op1=AL.add)
                    dendw = mid.tile([128, NLE // 16], dt.int16, tag="dendw")
                    nc.sync.dma_start(dendw[:], pd[f"dendw{h}"][:])
                    for j in range(NJ):
                        a, b = j * 512, min(NLE, (j + 1) * 512)
                        jw = (b - a) // 16 if (b - a) % 16 == 0 else None
                        dgs = work1.tile([128, 512], dt.float32, tag="alo")
                        nc.gpsimd.ap_gather(
                            dgs[:, :b - a], exs[:],
                            dendw[:, a // 16:(a + (b - a)) // 16],
                            channels=128, num_elems=4096, d=1,
                            num_idxs=b - a)
                        pdn = psum.tile([2, 512], dt.float32, tag="pw")
                        nc.tensor.matmul(pdn[:, :b - a], DSELt[:],
                                         dgs[:, :b - a],
                                         start=True, stop=True)
                        nc.vector.tensor_tensor(dsum[:, a:b], dsum[:, a:b],
                                                pdn[:, :b - a], AL.add)
                # --- C: main gather + msgs ---
                wxtab = big.tile([128, CHUNK], dt.float32, tag="big")
                for q in range(4):
                    rk = 4 * pc + q
                    nc.sync.dma_start(
                        wxtab[:, q * NLOC:(q + 1) * NLOC],
                        agout[rk * 144:rk * 144 + 128, :])
                for q in range(4):
                    scanq = res.tile([128, PQTR], dt.float32, tag="sh16")
                    for tt in range(2):
                        t = 2 * q + tt
                        exfm = res.tile([65, BLK], dt.bfloat16, tag="alv")
                        nc.sync.dma_start(exfm[0:1, :], ex8[16 * t:16 * t + 1, :])
                        nc.sync.dma_start(exfm[64:65, :],
                                          ex8[16 * t + 4:16 * t + 5, :])
                        for s in range(BLK // SUBM):
                            e0 = t * BLK + s * SUBM
                            w0 = e0 // 16
                            sw = SUBM // 16
                            colws = work.tile([128, SUBM // 16], dt.int16,
                                              tag="colws")
                            nc.sync.dma_start(colws[:],
                                              pd["colw"][:, w0:w0 + sw])
                            gt = work1.tile([128, SUBM // 4, 4], dt.float32,
                                           tag="gt")
                            nc.gpsimd.ap_gather(
                                gt[:], wxtab[:], colws[:],
                                channels=128, num_elems=CHUNK, d=1,
                                num_idxs=SUBM)
                            gb = work1.tile([128, SUBM // 4, 4], dt.bfloat16,
                                           tag="gb")
                            g2i = gt[:].rearrange("p a b -> p (a b)")
                            g2o = gb[:].rearrange("p a b -> p (a b)")
                            c0 = s * SUBM
                            exrep = psum1.tile([128, SUBM], dt.float32,
                                               tag="exrep")
                            for v2 in range(SUBM // 512):
                                va = v2 * 512
                                nc.tensor.matmul(
                                    exrep[0:64, va:va + 512], ones65[0:1, :],
                                    exfm[0:1, c0 + va:c0 + va + 512],
                                    start=True, stop=True)
                                nc.tensor.matmul(
                                    exrep[64:128, va:va + 512],
                                    ones65[64:65, :],
                                    exfm[64:65, c0 + va:c0 + va + 512],
                                    start=True, stop=True)
                            nc.vector.tensor_tensor(g2o[:, :], g2i[:, :],
                                                    exrep[:], AL.mult)
                            pp = psum1.tile([128, SUBM // 4], dt.float32,
                                            tag="pp")
                            nc.vector.tensor_reduce(
                                pp[:], gb[:], axis=mybir.AxisListType.X,
                                op=AL.add)
                            pb = tt * PBLK + s * (SUBM // 4)
                            pglob = t * PBLK + s * (SUBM // 4)
                            mkrs = work.tile([128, SUBM // 4], dt.bfloat16,
                                             tag="mkrs")
                            nc.sync.dma_start(
                                mkrs[:],
                                pd["pmrep"][:, pglob:pglob + SUBM // 4])
                            init = 0.0 if s % 4 == 0 else scanq[:, pb - 1:pb]
                            nc.vector.tensor_tensor_scan(
                                scanq[:, pb:pb + SUBM // 4],
                                mkrs[:], pp[:],
                                init, op0=AL.mult, op1=AL.add)
                    endw = mid.tile([128, NLE // 16], dt.int16, tag="dendw")
                    nc.sync.dma_start(endw[:], pd[f"endq{q}"][:])
                    for j in range(NJ):
                        a, b = j * 512, min(NLE, (j + 1) * 512)
                        bb = min(b, NLOC)
                        ehs = work1.tile([128, 512], dt.float32, tag="alo")
                        nc.gpsimd.ap_gather(
                            ehs[:, :b - a], scanq[:],
                            dendw[:, a // 16:b // 16] if False else
                            endw[:, a // 16:(a + (b - a)) // 16],
                            channels=128, num_elems=PQTR, d=1,
                            num_idxs=b - a)
                        if bb > a:
                            nc.vector.tensor_tensor(
                                msum[:, a:bb], msum[:, a:bb],
                                ehs[:, :bb - a], AL.add)

            # ---------- stage 4 ----------
            nc.vector.tensor_scalar(dsum[:], dsum[:], 1e-8, None, AL.add)
            drec = dsum
            nc.vector.reciprocal(drec[:], dsum[:])
            dsum65 = res.tile([65, NLE], dt.bfloat16, tag="sh16")
            nc.sync.dma_start(dsum65[0:1, :], drec[0:1, :])
            nc.sync.dma_start(dsum65[64:65, :], drec[1:2, :])
            msb = res.tile([128, NLOC], dt.bfloat16, tag="alv")
            for j in range(NT512):
                a, b = j * 512, min(NLOC, (j + 1) * 512)
                drep = psum1.tile([128, 512], dt.float32, tag="exrep")
                nc.tensor.matmul(drep[0:64, :b - a], ones65[0:1, :],
                                 dsum65[0:1, a:b], start=True, stop=True)
                nc.tensor.matmul(drep[64:128, :b - a], ones65[64:65, :],
                                 dsum65[64:65, a:b], start=True, stop=True)
                nc.vector.tensor_tensor(msb[:, a:b], msum[:, a:b],
                                        drep[:, :b - a], AL.mult)
            xTr = res.tile([128, NLOC], dt.bfloat16, tag="ex8")
            nc.sync.dma_start(xTr[:], xT_d[:])
            WRTt = mid.tile([128, 64], dt.bfloat16, tag="w128")
            WRBt = mid.tile([128, 64], dt.bfloat16, tag="w128b")
            nc.sync.dma_start(WRTt[:], WRT_d[:])
            nc.sync.dma_start(WRBt[:], WRB_d[:])
            osb = res.tile([64, NLOC], dt.float32, tag="sh16")
            for j in range(NT512):
                a, b = j * 512, min(NLOC, (j + 1) * 512)
                pr = psum.tile([64, 512], dt.float32, tag="pw")
                nc.tensor.matmul(pr[:, :b - a], WRTt[:], xTr[:, a:b],
                                 start=True, stop=False)
                nc.tensor.matmul(pr[:, :b - a], WRBt[:], msb[:, a:b],
                                 start=False, stop=True)
                et = work1.tile([64, 512], dt.float32, tag="gt")
                nc.scalar.activation(et[:, :b - a], pr[:, :b - a], AF.Exp)
                nc.vector.tensor_scalar(et[:, :b - a], et[:, :b - a],
                                        -1.0, 0.0, AL.add, AL.min)
                nc.vector.tensor_scalar(pr[:, :b - a], pr[:, :b - a],
                                        0.0, None, AL.max)
                nc.vector.tensor_tensor(osb[:, a:b], et[:, :b - a],
                                        pr[:, :b - a], AL.add)
            nc.sync.dma_start(out_d[:], osb[:])

    nc.compile()
    return nc


_CACHED = {}


def kernel(**inputs):
    import sys
    if '/opt/trn_rl_repo' not in sys.path:
        sys.path.insert(0, '/opt/trn_rl_repo')
    from concourse import bass_utils

    np_inputs = {k: np.asarray(v) for k, v in inputs.items()}
    in_maps = host_prep(**np_inputs)
    if 'nc' not in _CACHED:
        _CACHED['nc'] = build_bass()
    nc = _CACHED['nc']
    res = bass_utils.run_bass_kernel_spmd(nc, in_maps,
                                          core_ids=list(range(NC)))
    outs = [res.results[m]["out"] for m in range(NC)]
    return np.concatenate([o.T for o in outs], axis=0).astype(np.float32)

